# revision 20
# baseline (speedup 1.0000x reference)
"""Linear-attention Trainium2 Bass kernel (v6).

Reference computation (per batch b, head h):
    qkv = x @ W^T; q,k,v -> (h, t, 64)
    k masked rows -> -inf; prepend 4 mem-kv rows
    q = softmax(q * d^-0.5, axis=feature); k = softmax(k, axis=sequence)
    ctx = k^T v (64x64); out = q @ ctx; out *= mask

Optimizations:
  - Token compaction: masked tokens (~50%) contribute nothing to the
    output (out *= mask) nor to k-softmax (weight 0). Host gathers
    unmasked tokens per batch, pads to T (multiple of 512, adapted to
    the mask; program compiled per T and cached), scatters back.
  - q/k projections in fp8 e4m3 with DoubleRow perf mode (2x matmul
    throughput, contraction 256/pass). Weights scaled by 64 on host to
    dodge fp8 subnormals; 1/64 folded into the exp() activation scale.
    q-error is suppressed by the 1/8 softmax scale; k-error averages
    out across the sequence softmax. v stays bf16 (its weight-quant
    error is coherent across tokens and would dominate).
  - All other operands bf16 (PSUM fp32).
  - x streamed in 512-token chunks, packed host-side so both DRAM and
    SBUF sides of every DMA are contiguous 4-8KB lines per partition.
  - Pass-B: 4 matmuls per 128-token block (denominators fused via
    ones-columns of the context matrix), one reciprocal + one
    broadcast multiply per half-bank; 6 PSUM banks in flight.

Sharding: 8 cores = (batch 0..3) x (head-half 0..1); each core owns one
batch and 8 heads (4 head-pairs). No cross-core communication.
"""

import numpy as np
import ml_dtypes

D_MODEL = 1024
N_HEADS = 16
D_HEAD = 64
NMEM = 4
SCALE = D_HEAD ** -0.5
B = 4
L = 4096
NCORES = 8
HPC = 8            # heads per core
NPAIR = HPC // 2   # head-pairs per core
ECOLS = HPC * D_HEAD  # 512 output columns per core

CHUNK = 512        # tokens per x-DMA chunk / q-projection matmul
WSCALE = 64.0      # fp8 weight pre-scale (dodges e4m3 subnormals)

BF16 = ml_dtypes.bfloat16
F8 = ml_dtypes.float8_e4m3

_CACHE = {}


def build_nc(seqlen):
    """Per-core Bass program (identical across cores; data differs)."""
    import concourse.tile as tile
    from concourse import bacc, mybir

    f32 = mybir.dt.float32
    bf16 = mybir.dt.bfloat16
    f8 = mybir.dt.float8e4
    AF = mybir.ActivationFunctionType
    MUL = mybir.AluOpType.mult
    DR = mybir.MatmulPerfMode.DoubleRow

    chunk = CHUNK
    assert seqlen % chunk == 0
    n_chunks = seqlen // chunk
    tb_per_chunk = chunk // 128
    n_tb = seqlen // 128
    NDB = D_MODEL // 128          # 8 bf16 contraction blocks
    NDB8 = D_MODEL // 256         # 4 fp8 DoubleRow contraction blocks

    nc = bacc.Bacc("TRN2", target_bir_lowering=False, debug=False)

    xT8 = nc.dram_tensor("xT8", (n_chunks, 128, NDB8 * 2 * chunk), f8, kind="ExternalInput").ap()
    xT16 = nc.dram_tensor("xT16", (n_chunks, 128, NDB * chunk), bf16, kind="ExternalInput").ap()
    wq8 = nc.dram_tensor("wq8", (128, NDB8 * 2 * ECOLS), f8, kind="ExternalInput").ap()
    wk8 = nc.dram_tensor("wk8", (128, NDB8 * 2 * ECOLS), f8, kind="ExternalInput").ap()
    wv16 = nc.dram_tensor("wv16", (128, NDB * ECOLS), bf16, kind="ExternalInput").ap()
    mkp = nc.dram_tensor("mkp", (NPAIR, NMEM, 128), bf16, kind="ExternalInput").ap()
    mvp = nc.dram_tensor("mvp", (NPAIR, NMEM, 130), bf16, kind="ExternalInput").ap()
    biasm = nc.dram_tensor("biasm", (128, n_tb), f32, kind="ExternalInput").ap()
    out = nc.dram_tensor("out", (seqlen, ECOLS), bf16, kind="ExternalOutput").ap()

    with tile.TileContext(nc) as tc:
        with (
            tc.tile_pool(name="const", bufs=1) as cpool,
            tc.tile_pool(name="big", bufs=1) as bigpool,
            tc.tile_pool(name="small", bufs=8) as small,
            tc.tile_pool(name="xt8p", bufs=3) as xt8_pool,
            tc.tile_pool(name="xt16p", bufs=3) as xt16_pool,
            tc.tile_pool(name="ek", bufs=3) as ek_pool,
            tc.tile_pool(name="vv", bufs=3) as vv_pool,
            tc.tile_pool(name="osb", bufs=4) as osb_pool,
        ):
            # ---- weights/x: wq + first x chunk first for earliest start ----
            wq_sb = cpool.tile([128, NDB8, 2, ECOLS], f8, name="wq_sb", tag="wq_sb")
            nc.sync.dma_start(out=wq_sb, in_=wq8.rearrange("p (db ko e) -> p db ko e", db=NDB8, ko=2))

            def load8(ch):
                x8 = xt8_pool.tile([128, NDB8, 2, chunk], f8, name="x8")
                nc.sync.dma_start(out=x8, in_=xT8[ch].rearrange("p (db ko t) -> p db ko t", db=NDB8, ko=2))
                return x8

            def load16(ch):
                x16 = xt16_pool.tile([128, NDB, chunk], bf16, name="x16")
                nc.sync.dma_start(out=x16, in_=xT16[ch].rearrange("p (db t) -> p db t", db=NDB))
                return x16

            xt8_0 = load8(0)

            wk_sb = cpool.tile([128, NDB8, 2, ECOLS], f8, name="wk_sb", tag="wk_sb")
            nc.sync.dma_start(out=wk_sb, in_=wk8.rearrange("p (db ko e) -> p db ko e", db=NDB8, ko=2))

            xt8_1 = load8(1) if n_chunks > 1 else None
            xt16_0 = load16(0)

            wv_sb = cpool.tile([128, NDB, ECOLS], bf16, name="wv_sb", tag="wv_sb")
            nc.sync.dma_start(out=wv_sb, in_=wv16.rearrange("p (db e) -> p db e", db=NDB))

            mk_sb = cpool.tile([NMEM, NPAIR * 128], bf16, name="mk_sb", tag="mk_sb")
            nc.sync.dma_start(
                out=mk_sb.rearrange("n (g d) -> n g d", g=NPAIR),
                in_=mkp.rearrange("g n d -> n g d"),
            )
            mv_sb = cpool.tile([NMEM, NPAIR * 130], bf16, name="mv_sb", tag="mv_sb")
            nc.sync.dma_start(
                out=mv_sb.rearrange("n (g e) -> n g e", g=NPAIR),
                in_=mvp.rearrange("g n e -> n g e"),
            )
            expmk_sb = cpool.tile([NMEM, NPAIR * 128], bf16, name="expmk_sb", tag="expmk_sb")
            nc.scalar.activation(expmk_sb, mk_sb, AF.Exp)

            biasm_sb = cpool.tile([128, n_tb], f32, name="biasm_sb", tag="biasm_sb")
            nc.sync.dma_start(out=biasm_sb, in_=biasm)

            # exp(q * scale) for the whole batch, kept resident: [128, pair, t]
            expq_sb = bigpool.tile([128, NPAIR, seqlen], bf16, name="expq_sb", tag="expq_sb")

            # ctxbd zeros + ones columns set up front (DVE is idle here);
            # finalize only writes the normalized ctx blocks
            ctxbd = cpool.tile([128, NPAIR * 130], bf16, name="ctxbd", tag="ctxbd")
            nc.vector.memset(ctxbd, 0.0)
            for g in range(NPAIR):
                o = g * 130
                nc.vector.memset(ctxbd[0:64, o + 64 : o + 65], 1.0)
                nc.vector.memset(ctxbd[64:128, o + 129 : o + 130], 1.0)

            with tc.tile_pool(name="ctxps", bufs=1, space="PSUM") as ctx_pool:
                # persistent context accumulators: one bank per pair
                ctx_ps = [
                    ctx_pool.tile([128, 130], f32, name=f"ctx_ps{i}", tag=f"ctx{i}")
                    for i in range(NPAIR)
                ]

                # mem-kv rows initialize the context accumulation
                for g in range(NPAIR):
                    nc.tensor.matmul(
                        ctx_ps[g],
                        lhsT=expmk_sb[:, g * 128 : (g + 1) * 128],
                        rhs=mv_sb[:, g * 130 : (g + 1) * 130],
                        start=True,
                        stop=False,
                    )

                # ---- pass A: projection + exp(k) + context accumulation ----
                with (
                    tc.tile_pool(name="pq", bufs=2, space="PSUM") as pq_pool,
                    tc.tile_pool(name="pk", bufs=1, space="PSUM") as pk_pool,
                    tc.tile_pool(name="pv", bufs=1, space="PSUM") as pv_pool,
                ):
                    x8_tiles = {0: xt8_0}
                    if xt8_1 is not None:
                        x8_tiles[1] = xt8_1
                    x16_tiles = {0: xt16_0}

                    def get8(ch):
                        if ch not in x8_tiles:
                            x8_tiles[ch] = load8(ch)
                        return x8_tiles[ch]

                    def get16(ch):
                        if ch not in x16_tiles:
                            x16_tiles[ch] = load16(ch)
                        return x16_tiles[ch]

                    def emit_q(ch):
                        c0 = ch * chunk
                        xt8 = get8(ch)
                        # qT projection per pair (fp8 DoubleRow, K=256/pass)
                        for g in range(NPAIR):
                            pq = pq_pool.tile([128, chunk], f32, name="pq")
                            for db in range(NDB8):
                                nc.tensor.matmul(
                                    pq,
                                    lhsT=wq_sb[:, db, :, g * 128 : (g + 1) * 128],
                                    rhs=xt8[:, db, :, :],
                                    start=(db == 0),
                                    stop=(db == NDB8 - 1),
                                    perf_mode=DR,
                                )
                            nc.scalar.activation(
                                expq_sb[:, g, c0 : c0 + chunk], pq, AF.Exp,
                                scale=SCALE / WSCALE,
                            )

                    def emit_kv(ch):
                        xt8 = get8(ch)
                        xt16 = get16(ch)
                        # k (fp8 DoubleRow) / v (bf16) + exp(k)+mask + context
                        for tbi in range(tb_per_chunk):
                            j = ch * tb_per_chunk + tbi
                            pk = pk_pool.tile([128, ECOLS], f32, name="pk")
                            pv = pv_pool.tile([128, ECOLS], f32, name="pv")
                            for db in range(NDB8):
                                nc.tensor.matmul(
                                    pk,
                                    lhsT=xt8[:, db, :, tbi * 128 : (tbi + 1) * 128],
                                    rhs=wk_sb[:, db, :, :],
                                    start=(db == 0),
                                    stop=(db == NDB8 - 1),
                                    perf_mode=DR,
                                )
                            for db in range(NDB):
                                nc.tensor.matmul(
                                    pv,
                                    lhsT=xt16[:, db, tbi * 128 : (tbi + 1) * 128],
                                    rhs=wv_sb[:, db, :],
                                    start=(db == 0),
                                    stop=(db == NDB - 1),
                                )
                            ek = ek_pool.tile([128, ECOLS], bf16, name="ek")
                            nc.scalar.activation(
                                ek, pk, AF.Exp, bias=biasm_sb[:, j : j + 1],
                                scale=1.0 / WSCALE,
                            )
                            vv = vv_pool.tile([128, NPAIR * 130], bf16, name="vv")
                            vv_g = vv.rearrange("p (g e) -> p g e", g=NPAIR)
                            nc.vector.tensor_copy(
                                vv_g[:, :, 0:128],
                                pv.rearrange("p (g e) -> p g e", g=NPAIR),
                            )
                            nc.gpsimd.memset(vv_g[:, :, 128:130], 1.0)
                            for g in range(NPAIR):
                                nc.tensor.matmul(
                                    ctx_ps[g],
                                    lhsT=ek[:, g * 128 : (g + 1) * 128],
                                    rhs=vv[:, g * 130 : (g + 1) * 130],
                                    start=False,
                                    stop=(j == n_tb - 1),
                                )

                    # q for chunk ch+1 ahead of k/v for chunk ch: q needs
                    # only xt8, so the PE has work while wv/xt16 stream in
                    emit_q(0)
                    for ch in range(n_chunks):
                        if ch + 1 < n_chunks:
                            emit_q(ch + 1)
                        emit_kv(ch)

                # ---- finalize: normalize context, build block-diag rhs ----
                # per pair g (130 cols): [h0 ctx 64 | ones | h1 ctx 64 | ones]
                # (zeros/ones of ctxbd were set at kernel start)
                for g in range(NPAIR):
                    ps = ctx_ps[g]
                    rk = small.tile([128, 1], f32, name="rk", tag="rk")
                    nc.vector.reciprocal(rk, ps[:, 128:129])
                    o = g * 130
                    nc.vector.tensor_scalar_mul(
                        ctxbd[0:64, o : o + 64], ps[0:64, 0:64], rk[0:64]
                    )
                    nc.vector.tensor_scalar_mul(
                        ctxbd[64:128, o + 65 : o + 129], ps[64:128, 64:128], rk[64:128]
                    )

            # ---- pass B: out = exp_q @ ctx, fused normalization ----
            # po bank layout (260 cols = 2 pairs): per pair
            #   [h0 out 64 | h0 den 1 | h1 out 64 | h1 den 1]
            with (
                tc.tile_pool(name="po", bufs=6, space="PSUM") as po_pool,
                tc.tile_pool(name="pos", bufs=4) as pos_pool,
            ):
                for j in range(n_tb):
                    t0 = j * 128
                    osb = osb_pool.tile([128, ECOLS], bf16, name="osb")
                    for half in range(2):
                        po = po_pool.tile([128, 260], f32, name="po")
                        for gi in range(2):
                            g = half * 2 + gi
                            nc.tensor.matmul(
                                po[:, gi * 130 : (gi + 1) * 130],
                                lhsT=expq_sb[:, g, t0 : t0 + 128],
                                rhs=ctxbd[:, g * 130 : (g + 1) * 130],
                                start=(gi == 0),
                                stop=(gi == 1),
                            )
                        # po viewed as [p, pair, head, 65]: den at col 64
                        po4 = po.rearrange("p (g h c) -> p g h c", g=2, h=2)
                        rq = small.tile([128, 4], f32, name="rq", tag=f"rq{half}")
                        rq4 = rq.rearrange("p (g h o) -> p g h o", g=2, h=2)
                        nc.vector.reciprocal(rq4, po4[:, :, :, 64:65])
                        ob4 = osb[:, half * 256 : (half + 1) * 256].rearrange(
                            "p (g h e) -> p g h e", g=2, h=2
                        )
                        if half == 0:
                            nc.vector.tensor_tensor(
                                ob4,
                                po4[:, :, :, 0:64],
                                rq4.to_broadcast(ob4.shape),
                                MUL,
                            )
                        else:
                            # GpSimd cannot read PSUM: stage via ScalarE in
                            # f32 (bf16 staging would double-round the output)
                            pos = pos_pool.tile([128, 256], f32, name="pos")
                            pos4 = pos.rearrange("p (g h e) -> p g h e", g=2, h=2)
                            nc.scalar.activation(pos4, po4[:, :, :, 0:64], AF.Copy)
                            nc.gpsimd.tensor_tensor(
                                ob4,
                                pos4,
                                rq4.to_broadcast(ob4.shape),
                                MUL,
                            )
                    nc.sync.dma_start(out=out[t0 : t0 + 128, :], in_=osb)

    nc.compile()
    return nc


def _host_inputs(x, w_qkv, mem_kv, mask, seqlen):
    """Compact (gather unmasked tokens), pad, pack, cast."""
    x = np.asarray(x, dtype=np.float32)
    w_qkv = np.asarray(w_qkv, dtype=np.float32)
    mem_kv = np.asarray(mem_kv, dtype=np.float32)
    mask = np.asarray(mask)

    nb = x.shape[0]
    n_ch = seqlen // CHUNK
    idxs = [np.nonzero(mask[b])[0] for b in range(nb)]

    def pack8(aT):       # [1024, T] uint8 -> [n_ch, 128, 4*2*CHUNK]
        a = aT.reshape(4, 2, 128, seqlen).transpose(2, 0, 1, 3)
        a = a.reshape(128, 8, n_ch, CHUNK).transpose(2, 0, 1, 3)
        return np.ascontiguousarray(a.reshape(n_ch, 128, 8 * CHUNK))

    def pack16(aT):      # [1024, T] uint16 -> [n_ch, 128, 8*CHUNK]
        a = aT.reshape(8, 128, seqlen).transpose(1, 0, 2)
        a = a.reshape(128, 8, n_ch, CHUNK).transpose(2, 0, 1, 3)
        return np.ascontiguousarray(a.reshape(n_ch, 128, 8 * CHUNK))

    xT8, xT16 = [], []
    for b in range(nb):
        xc = np.zeros((seqlen, D_MODEL), np.float32)
        xc[: len(idxs[b])] = x[b, idxs[b]]
        xcT = np.ascontiguousarray(xc.T)
        xT8.append(pack8(xcT.astype(F8).view(np.uint8)).view(F8))
        xT16.append(pack16(xcT.astype(BF16).view(np.uint16)).view(BF16))

    w4 = w_qkv.reshape(N_HEADS, D_HEAD, 3, D_MODEL)
    wT = {}
    for half in (0, 1):
        h0 = half * HPC
        for ci, cn in ((0, "q"), (1, "k"), (2, "v")):
            wT[(half, cn)] = np.ascontiguousarray(
                w4[h0 : h0 + HPC, :, ci, :].reshape(ECOLS, D_MODEL).T
            )

    def packw8(wTa):     # [1024, 512] -> [128, 4*2*512] fp8, x64 scaled
        a = (wTa * WSCALE).astype(F8).view(np.uint8)
        a = a.reshape(4, 2, 128, ECOLS).transpose(2, 0, 1, 3)
        return np.ascontiguousarray(a.reshape(128, 8 * ECOLS)).view(F8)

    def packw16(wTa):    # [1024, 512] -> [128, 8*512] bf16
        a = wTa.astype(BF16).view(np.uint16)
        a = a.reshape(8, 128, ECOLS).transpose(1, 0, 2)
        return np.ascontiguousarray(a.reshape(128, 8 * ECOLS)).view(BF16)

    n_tb = seqlen // 128
    in_maps = []
    for c in range(NCORES):
        b, half = divmod(c, 2)
        h0 = half * HPC
        mk = (
            mem_kv[0, h0 : h0 + HPC]
            .reshape(NPAIR, 2, NMEM, D_HEAD)
            .transpose(0, 2, 1, 3)
            .reshape(NPAIR, NMEM, 128)
        )
        mv = (
            mem_kv[1, h0 : h0 + HPC]
            .reshape(NPAIR, 2, NMEM, D_HEAD)
            .transpose(0, 2, 1, 3)
            .reshape(NPAIR, NMEM, 128)
        )
        # ctx bank layout: cols 0:64 h0-v-feats, 64:128 h1-v-feats, 128:130 ones
        mvp = np.ones((NPAIR, NMEM, 130), np.float32)
        mvp[:, :, :128] = mv
        cnt = len(idxs[b])
        mfb = np.zeros(seqlen, np.float32)
        mfb[:cnt] = 1.0
        biasm = np.ascontiguousarray(((mfb - 1.0) * 1e30).reshape(n_tb, 128).T)
        in_maps.append(
            {
                "xT8": xT8[b],
                "xT16": xT16[b],
                "wq8": packw8(wT[(half, "q")]),
                "wk8": packw8(wT[(half, "k")]),
                "wv16": packw16(wT[(half, "v")]),
                "mkp": np.ascontiguousarray(mk).astype(BF16),
                "mvp": mvp.astype(BF16),
                "biasm": biasm,
            }
        )
    return in_maps, idxs


def _get_nc(seqlen):
    key = ("nc", seqlen)
    if key not in _CACHE:
        _CACHE[key] = build_nc(seqlen)
    return _CACHE[key]


def _pick_seqlen(mask):
    maxcnt = int(np.asarray(mask).sum(axis=1).max())
    return max(-(-maxcnt // CHUNK) * CHUNK, CHUNK)


def run(x, w_qkv, mem_kv, mask, **spmd_kwargs):
    from concourse.bass_utils import run_bass_kernel_spmd

    seqlen = _pick_seqlen(mask)
    nc = _get_nc(seqlen)
    in_maps, idxs = _host_inputs(x, w_qkv, mem_kv, mask, seqlen)
    res = run_bass_kernel_spmd(nc, in_maps, core_ids=list(range(NCORES)), **spmd_kwargs)
    out = np.zeros(np.asarray(x).shape[:2] + (D_MODEL,), np.float32)
    for c in range(NCORES):
        b, half = divmod(c, 2)
        cnt = len(idxs[b])
        out[b, idxs[b], half * ECOLS : (half + 1) * ECOLS] = (
            res.results[c]["out"][:cnt].astype(np.float32)
        )
    return out, res


def kernel(x, w_qkv, mem_kv, mask):
    out, _ = run(x, w_qkv, mem_kv, mask)
    return out


# revision 21
# speedup vs baseline: 1.0153x; 1.0153x over previous
"""Linear-attention Trainium2 Bass kernel.

Reference computation (per batch b, head h):
    qkv = x @ W^T; q,k,v -> (h, t, 64)
    k masked rows -> -inf; prepend 4 mem-kv rows
    q = softmax(q * d^-0.5, axis=feature); k = softmax(k, axis=sequence)
    ctx = k^T v (64x64); out = q @ ctx; out *= mask

Optimizations:
  - Token compaction: masked tokens (~50%) contribute nothing to the
    output (out *= mask) nor to k-softmax (weight 0). Host gathers
    unmasked tokens per batch, pads to T (multiple of 512, adapted to
    the mask; program compiled per T and cached), scatters back.
  - q/k projections in fp8 e4m3 with DoubleRow perf mode (2x matmul
    throughput, contraction 256/pass). Weights scaled by 64 on host to
    dodge fp8 subnormals; 1/64 folded into the exp() activation scale.
    q-error is suppressed by the 1/8 softmax scale; k-error averages
    out across the sequence softmax. v stays bf16 (its weight-quant
    error is coherent across tokens and would dominate).
  - All other operands bf16 (PSUM fp32).
  - x streamed in 512-token chunks, packed host-side so both DRAM and
    SBUF sides of every DMA are contiguous 4-8KB lines per partition.
  - Pass-B: 4 matmuls per 128-token block (denominators fused via
    ones-columns of the context matrix), one reciprocal + one
    broadcast multiply per half-bank; 6 PSUM banks in flight.

Sharding: 8 cores = (batch 0..3) x (head-half 0..1); each core owns one
batch and 8 heads (4 head-pairs). No cross-core communication.
"""

import numpy as np
import ml_dtypes

D_MODEL = 1024
N_HEADS = 16
D_HEAD = 64
NMEM = 4
SCALE = D_HEAD ** -0.5
B = 4
L = 4096
NCORES = 8
HPC = 8            # heads per core
NPAIR = HPC // 2   # head-pairs per core
ECOLS = HPC * D_HEAD  # 512 output columns per core

CHUNK = 512        # tokens per x-DMA chunk / q-projection matmul
WSCALE = 64.0      # fp8 weight pre-scale (dodges e4m3 subnormals)

BF16 = ml_dtypes.bfloat16
F8 = ml_dtypes.float8_e4m3

_CACHE = {}


def build_nc(seqlen):
    """Per-core Bass program (identical across cores; data differs)."""
    import concourse.tile as tile
    from concourse import bacc, mybir

    f32 = mybir.dt.float32
    bf16 = mybir.dt.bfloat16
    f8 = mybir.dt.float8e4
    AF = mybir.ActivationFunctionType
    MUL = mybir.AluOpType.mult
    DR = mybir.MatmulPerfMode.DoubleRow

    chunk = CHUNK
    assert seqlen % chunk == 0
    n_chunks = seqlen // chunk
    tb_per_chunk = chunk // 128
    n_tb = seqlen // 128
    NDB = D_MODEL // 128          # 8 bf16 contraction blocks
    NDB8 = D_MODEL // 256         # 4 fp8 DoubleRow contraction blocks

    nc = bacc.Bacc("TRN2", target_bir_lowering=False, debug=False)

    xT8 = nc.dram_tensor("xT8", (n_chunks, 128, NDB8 * 2 * chunk), f8, kind="ExternalInput").ap()
    xT16 = nc.dram_tensor("xT16", (n_chunks, 128, NDB * chunk), bf16, kind="ExternalInput").ap()
    wq8 = nc.dram_tensor("wq8", (128, NDB8 * 2 * ECOLS), f8, kind="ExternalInput").ap()
    wk8 = nc.dram_tensor("wk8", (128, NDB8 * 2 * ECOLS), f8, kind="ExternalInput").ap()
    wv16 = nc.dram_tensor("wv16", (128, NDB * ECOLS), bf16, kind="ExternalInput").ap()
    mkp = nc.dram_tensor("mkp", (NPAIR, NMEM, 128), bf16, kind="ExternalInput").ap()
    mvp = nc.dram_tensor("mvp", (NPAIR, NMEM, 130), bf16, kind="ExternalInput").ap()
    biasm = nc.dram_tensor("biasm", (128, n_tb), f32, kind="ExternalInput").ap()
    out = nc.dram_tensor("out", (seqlen, ECOLS), bf16, kind="ExternalOutput").ap()

    with tile.TileContext(nc) as tc:
        with (
            tc.tile_pool(name="const", bufs=1) as cpool,
            tc.tile_pool(name="big", bufs=1) as bigpool,
            tc.tile_pool(name="small", bufs=8) as small,
            tc.tile_pool(name="xt8p", bufs=3) as xt8_pool,
            tc.tile_pool(name="xt16p", bufs=3) as xt16_pool,
            tc.tile_pool(name="ek", bufs=3) as ek_pool,
            tc.tile_pool(name="vv", bufs=3) as vv_pool,
            tc.tile_pool(name="osb", bufs=4) as osb_pool,
        ):
            # ---- weights/x: wq + first x chunk first for earliest start ----
            wq_sb = cpool.tile([128, NDB8, 2, ECOLS], f8, name="wq_sb", tag="wq_sb")
            nc.sync.dma_start(out=wq_sb, in_=wq8.rearrange("p (db ko e) -> p db ko e", db=NDB8, ko=2))

            def load8(ch):
                x8 = xt8_pool.tile([128, NDB8, 2, chunk], f8, name="x8")
                nc.sync.dma_start(out=x8, in_=xT8[ch].rearrange("p (db ko t) -> p db ko t", db=NDB8, ko=2))
                return x8

            def load16(ch):
                x16 = xt16_pool.tile([128, NDB, chunk], bf16, name="x16")
                nc.sync.dma_start(out=x16, in_=xT16[ch].rearrange("p (db t) -> p db t", db=NDB))
                return x16

            xt8_0 = load8(0)

            wk_sb = cpool.tile([128, NDB8, 2, ECOLS], f8, name="wk_sb", tag="wk_sb")
            nc.sync.dma_start(out=wk_sb, in_=wk8.rearrange("p (db ko e) -> p db ko e", db=NDB8, ko=2))

            xt8_1 = load8(1) if n_chunks > 1 else None
            xt16_0 = load16(0)

            wv_sb = cpool.tile([128, NDB, ECOLS], bf16, name="wv_sb", tag="wv_sb")
            nc.sync.dma_start(out=wv_sb, in_=wv16.rearrange("p (db e) -> p db e", db=NDB))

            mk_sb = cpool.tile([NMEM, NPAIR * 128], bf16, name="mk_sb", tag="mk_sb")
            nc.sync.dma_start(
                out=mk_sb.rearrange("n (g d) -> n g d", g=NPAIR),
                in_=mkp.rearrange("g n d -> n g d"),
            )
            mv_sb = cpool.tile([NMEM, NPAIR * 130], bf16, name="mv_sb", tag="mv_sb")
            nc.sync.dma_start(
                out=mv_sb.rearrange("n (g e) -> n g e", g=NPAIR),
                in_=mvp.rearrange("g n e -> n g e"),
            )
            expmk_sb = cpool.tile([NMEM, NPAIR * 128], bf16, name="expmk_sb", tag="expmk_sb")
            nc.scalar.activation(expmk_sb, mk_sb, AF.Exp)

            biasm_sb = cpool.tile([128, n_tb], f32, name="biasm_sb", tag="biasm_sb")
            nc.sync.dma_start(out=biasm_sb, in_=biasm)

            # exp(q * scale) for the whole batch, kept resident: [128, pair, t]
            expq_sb = bigpool.tile([128, NPAIR, seqlen], bf16, name="expq_sb", tag="expq_sb")

            # ctxbd zeros + ones columns set up front (DVE is idle here);
            # finalize only writes the normalized ctx blocks
            ctxbd = cpool.tile([128, NPAIR * 130], bf16, name="ctxbd", tag="ctxbd")
            nc.vector.memset(ctxbd, 0.0)
            for g in range(NPAIR):
                o = g * 130
                nc.vector.memset(ctxbd[0:64, o + 64 : o + 65], 1.0)
                nc.vector.memset(ctxbd[64:128, o + 129 : o + 130], 1.0)

            with tc.tile_pool(name="ctxps", bufs=1, space="PSUM") as ctx_pool:
                # persistent context accumulators: one bank per pair
                ctx_ps = [
                    ctx_pool.tile([128, 130], f32, name=f"ctx_ps{i}", tag=f"ctx{i}")
                    for i in range(NPAIR)
                ]

                # mem-kv rows initialize the context accumulation
                for g in range(NPAIR):
                    nc.tensor.matmul(
                        ctx_ps[g],
                        lhsT=expmk_sb[:, g * 128 : (g + 1) * 128],
                        rhs=mv_sb[:, g * 130 : (g + 1) * 130],
                        start=True,
                        stop=False,
                    )

                # ---- pass A: projection + exp(k) + context accumulation ----
                with (
                    tc.tile_pool(name="pq", bufs=2, space="PSUM") as pq_pool,
                    tc.tile_pool(name="pk", bufs=1, space="PSUM") as pk_pool,
                    tc.tile_pool(name="pv", bufs=1, space="PSUM") as pv_pool,
                ):
                    x8_tiles = {0: xt8_0}
                    if xt8_1 is not None:
                        x8_tiles[1] = xt8_1
                    x16_tiles = {0: xt16_0}

                    def get8(ch):
                        if ch not in x8_tiles:
                            x8_tiles[ch] = load8(ch)
                        return x8_tiles[ch]

                    def get16(ch):
                        if ch not in x16_tiles:
                            x16_tiles[ch] = load16(ch)
                        return x16_tiles[ch]

                    def emit_q(ch):
                        c0 = ch * chunk
                        xt8 = get8(ch)
                        # qT projection per pair (fp8 DoubleRow, K=256/pass)
                        for g in range(NPAIR):
                            pq = pq_pool.tile([128, chunk], f32, name="pq")
                            for db in range(NDB8):
                                nc.tensor.matmul(
                                    pq,
                                    lhsT=wq_sb[:, db, :, g * 128 : (g + 1) * 128],
                                    rhs=xt8[:, db, :, :],
                                    start=(db == 0),
                                    stop=(db == NDB8 - 1),
                                    perf_mode=DR,
                                )
                            nc.scalar.activation(
                                expq_sb[:, g, c0 : c0 + chunk], pq, AF.Exp,
                                scale=SCALE / WSCALE,
                            )

                    def emit_kv(ch):
                        xt8 = get8(ch)
                        xt16 = get16(ch)
                        # k (fp8 DoubleRow) / v (bf16) + exp(k)+mask + context
                        for tbi in range(tb_per_chunk):
                            j = ch * tb_per_chunk + tbi
                            pk = pk_pool.tile([128, ECOLS], f32, name="pk")
                            pv = pv_pool.tile([128, ECOLS], f32, name="pv")
                            for db in range(NDB8):
                                nc.tensor.matmul(
                                    pk,
                                    lhsT=xt8[:, db, :, tbi * 128 : (tbi + 1) * 128],
                                    rhs=wk_sb[:, db, :, :],
                                    start=(db == 0),
                                    stop=(db == NDB8 - 1),
                                    perf_mode=DR,
                                )
                            for db in range(NDB):
                                nc.tensor.matmul(
                                    pv,
                                    lhsT=xt16[:, db, tbi * 128 : (tbi + 1) * 128],
                                    rhs=wv_sb[:, db, :],
                                    start=(db == 0),
                                    stop=(db == NDB - 1),
                                )
                            ek = ek_pool.tile([128, ECOLS], bf16, name="ek")
                            nc.scalar.activation(
                                ek, pk, AF.Exp, bias=biasm_sb[:, j : j + 1],
                                scale=1.0 / WSCALE,
                            )
                            vv = vv_pool.tile([128, NPAIR * 130], bf16, name="vv")
                            vv_g = vv.rearrange("p (g e) -> p g e", g=NPAIR)
                            nc.vector.tensor_copy(
                                vv_g[:, :, 0:128],
                                pv.rearrange("p (g e) -> p g e", g=NPAIR),
                            )
                            nc.gpsimd.memset(vv_g[:, :, 128:130], 1.0)
                            for g in range(NPAIR):
                                nc.tensor.matmul(
                                    ctx_ps[g],
                                    lhsT=ek[:, g * 128 : (g + 1) * 128],
                                    rhs=vv[:, g * 130 : (g + 1) * 130],
                                    start=False,
                                    stop=(j == n_tb - 1),
                                )

                    # q for chunk ch+1 ahead of k/v for chunk ch: q needs
                    # only xt8, so the PE has work while wv/xt16 stream in
                    emit_q(0)
                    for ch in range(n_chunks):
                        if ch + 1 < n_chunks:
                            emit_q(ch + 1)
                        emit_kv(ch)

                # ---- finalize: normalize context, build block-diag rhs ----
                # per pair g (130 cols): [h0 ctx 64 | ones | h1 ctx 64 | ones]
                # (zeros/ones of ctxbd were set at kernel start)
                for g in range(NPAIR):
                    ps = ctx_ps[g]
                    rk = small.tile([128, 1], f32, name="rk", tag="rk")
                    nc.vector.reciprocal(rk, ps[:, 128:129])
                    o = g * 130
                    nc.vector.tensor_scalar_mul(
                        ctxbd[0:64, o : o + 64], ps[0:64, 0:64], rk[0:64]
                    )
                    nc.vector.tensor_scalar_mul(
                        ctxbd[64:128, o + 65 : o + 129], ps[64:128, 64:128], rk[64:128]
                    )

            # ---- pass B: out = exp_q @ ctx, fused normalization ----
            # po bank layout (260 cols = 2 pairs): per pair
            #   [h0 out 64 | h0 den 1 | h1 out 64 | h1 den 1]
            with (
                tc.tile_pool(name="po", bufs=6, space="PSUM") as po_pool,
                tc.tile_pool(name="pos", bufs=4) as pos_pool,
            ):
                for j in range(n_tb):
                    t0 = j * 128
                    osb = osb_pool.tile([128, ECOLS], bf16, name="osb")
                    for half in range(2):
                        po = po_pool.tile([128, 260], f32, name="po")
                        for gi in range(2):
                            g = half * 2 + gi
                            nc.tensor.matmul(
                                po[:, gi * 130 : (gi + 1) * 130],
                                lhsT=expq_sb[:, g, t0 : t0 + 128],
                                rhs=ctxbd[:, g * 130 : (g + 1) * 130],
                                start=(gi == 0),
                                stop=(gi == 1),
                            )
                        # po viewed as [p, pair, head, 65]: den at col 64
                        po4 = po.rearrange("p (g h c) -> p g h c", g=2, h=2)
                        rq = small.tile([128, 4], f32, name="rq", tag=f"rq{half}")
                        rq4 = rq.rearrange("p (g h o) -> p g h o", g=2, h=2)
                        nc.vector.reciprocal(rq4, po4[:, :, :, 64:65])
                        ob4 = osb[:, half * 256 : (half + 1) * 256].rearrange(
                            "p (g h e) -> p g h e", g=2, h=2
                        )
                        if half == 0:
                            nc.vector.tensor_tensor(
                                ob4,
                                po4[:, :, :, 0:64],
                                rq4.to_broadcast(ob4.shape),
                                MUL,
                            )
                        else:
                            # GpSimd cannot read PSUM: stage via ScalarE in
                            # f32 (bf16 staging would double-round the output)
                            pos = pos_pool.tile([128, 256], f32, name="pos")
                            pos4 = pos.rearrange("p (g h e) -> p g h e", g=2, h=2)
                            nc.scalar.activation(pos4, po4[:, :, :, 0:64], AF.Copy)
                            nc.gpsimd.tensor_tensor(
                                ob4,
                                pos4,
                                rq4.to_broadcast(ob4.shape),
                                MUL,
                            )
                    nc.sync.dma_start(out=out[t0 : t0 + 128, :], in_=osb)

    nc.compile()
    return nc


def _host_inputs(x, w_qkv, mem_kv, mask, seqlen):
    """Compact (gather unmasked tokens), pad, pack, cast."""
    x = np.asarray(x, dtype=np.float32)
    w_qkv = np.asarray(w_qkv, dtype=np.float32)
    mem_kv = np.asarray(mem_kv, dtype=np.float32)
    mask = np.asarray(mask)

    nb = x.shape[0]
    n_ch = seqlen // CHUNK
    idxs = [np.nonzero(mask[b])[0] for b in range(nb)]

    def pack8(aT):       # [1024, T] uint8 -> [n_ch, 128, 4*2*CHUNK]
        a = aT.reshape(4, 2, 128, seqlen).transpose(2, 0, 1, 3)
        a = a.reshape(128, 8, n_ch, CHUNK).transpose(2, 0, 1, 3)
        return np.ascontiguousarray(a.reshape(n_ch, 128, 8 * CHUNK))

    def pack16(aT):      # [1024, T] uint16 -> [n_ch, 128, 8*CHUNK]
        a = aT.reshape(8, 128, seqlen).transpose(1, 0, 2)
        a = a.reshape(128, 8, n_ch, CHUNK).transpose(2, 0, 1, 3)
        return np.ascontiguousarray(a.reshape(n_ch, 128, 8 * CHUNK))

    xT8, xT16 = [], []
    for b in range(nb):
        xc = np.zeros((seqlen, D_MODEL), np.float32)
        xc[: len(idxs[b])] = x[b, idxs[b]]
        xcT = np.ascontiguousarray(xc.T)
        xT8.append(pack8(xcT.astype(F8).view(np.uint8)).view(F8))
        xT16.append(pack16(xcT.astype(BF16).view(np.uint16)).view(BF16))

    w4 = w_qkv.reshape(N_HEADS, D_HEAD, 3, D_MODEL)
    wT = {}
    for half in (0, 1):
        h0 = half * HPC
        for ci, cn in ((0, "q"), (1, "k"), (2, "v")):
            wT[(half, cn)] = np.ascontiguousarray(
                w4[h0 : h0 + HPC, :, ci, :].reshape(ECOLS, D_MODEL).T
            )

    def packw8(wTa):     # [1024, 512] -> [128, 4*2*512] fp8, x64 scaled
        a = (wTa * WSCALE).astype(F8).view(np.uint8)
        a = a.reshape(4, 2, 128, ECOLS).transpose(2, 0, 1, 3)
        return np.ascontiguousarray(a.reshape(128, 8 * ECOLS)).view(F8)

    def packw16(wTa):    # [1024, 512] -> [128, 8*512] bf16
        a = wTa.astype(BF16).view(np.uint16)
        a = a.reshape(8, 128, ECOLS).transpose(1, 0, 2)
        return np.ascontiguousarray(a.reshape(128, 8 * ECOLS)).view(BF16)

    n_tb = seqlen // 128
    in_maps = []
    for c in range(NCORES):
        b, half = divmod(c, 2)
        h0 = half * HPC
        mk = (
            mem_kv[0, h0 : h0 + HPC]
            .reshape(NPAIR, 2, NMEM, D_HEAD)
            .transpose(0, 2, 1, 3)
            .reshape(NPAIR, NMEM, 128)
        )
        mv = (
            mem_kv[1, h0 : h0 + HPC]
            .reshape(NPAIR, 2, NMEM, D_HEAD)
            .transpose(0, 2, 1, 3)
            .reshape(NPAIR, NMEM, 128)
        )
        # ctx bank layout: cols 0:64 h0-v-feats, 64:128 h1-v-feats, 128:130 ones
        mvp = np.ones((NPAIR, NMEM, 130), np.float32)
        mvp[:, :, :128] = mv
        cnt = len(idxs[b])
        mfb = np.zeros(seqlen, np.float32)
        mfb[:cnt] = 1.0
        biasm = np.ascontiguousarray(((mfb - 1.0) * 1e30).reshape(n_tb, 128).T)
        in_maps.append(
            {
                "xT8": xT8[b],
                "xT16": xT16[b],
                "wq8": packw8(wT[(half, "q")]),
                "wk8": packw8(wT[(half, "k")]),
                "wv16": packw16(wT[(half, "v")]),
                "mkp": np.ascontiguousarray(mk).astype(BF16),
                "mvp": mvp.astype(BF16),
                "biasm": biasm,
            }
        )
    return in_maps, idxs


def _get_nc(seqlen):
    key = ("nc", seqlen)
    if key not in _CACHE:
        _CACHE[key] = build_nc(seqlen)
    return _CACHE[key]


def _pick_seqlen(mask):
    maxcnt = int(np.asarray(mask).sum(axis=1).max())
    return max(-(-maxcnt // CHUNK) * CHUNK, CHUNK)


def run(x, w_qkv, mem_kv, mask, **spmd_kwargs):
    from concourse.bass_utils import run_bass_kernel_spmd

    seqlen = _pick_seqlen(mask)
    nc = _get_nc(seqlen)
    in_maps, idxs = _host_inputs(x, w_qkv, mem_kv, mask, seqlen)
    res = run_bass_kernel_spmd(nc, in_maps, core_ids=list(range(NCORES)), **spmd_kwargs)
    out = np.zeros(np.asarray(x).shape[:2] + (D_MODEL,), np.float32)
    for c in range(NCORES):
        b, half = divmod(c, 2)
        cnt = len(idxs[b])
        out[b, idxs[b], half * ECOLS : (half + 1) * ECOLS] = (
            res.results[c]["out"][:cnt].astype(np.float32)
        )
    return out, res


def kernel(x, w_qkv, mem_kv, mask):
    out, _ = run(x, w_qkv, mem_kv, mask)
    return out
